# revision 1
# baseline (speedup 1.0000x reference)
"""Causal self-attention (B=4, S=2048, D=768, H=12) on 8 trn2 NeuronCores.

Sharding (Megatron-style): DP over the 4 batches x TP=2 over heads.
Core c handles batch c//2 with heads (c%2)*6 .. +6: qkv_proj column-parallel,
out_proj row-parallel; the TP pair's partial outputs are summed on the host.

Per-core kernel, fp16 data path (fp32 PSUM accumulation everywhere):
  A. x arrives host-transposed as xT [d, s] fp16; weights fp16 (score scale
     1/sqrt(64) folded into Wq/bq on the host).
  B. qkT = (x @ Wqk)^T in [feat(part), s] layout (bias fused into the
     PSUM->SBUF copy on DVE); V in natural [s(part), feat] layout with a
     ones column (V') so PV also produces the softmax denominator.
  C. flash-style causal attention per (head, 512-q-chunk) job:
     S^T tile = K_tile @ Q^T (contraction 64), exp on ACT batched 2 k-tiles
     per ACTIVATE, causal diagonal via in-place affine_select on Pool;
     PV is FLIPPED: O[q(part), 65] += pt_chunk^T @ V' - 65-wide moving
     operand, so PV costs 65 rows/matmul instead of 512.  The denominator
     is then per-PARTITION: normalize = DVE reciprocal + per-q-tile
     tensor_scalar fused into the PSUM->SBUF copy.  Normalized O[q,64] is
     PE-transposed (identity matmul) into oT [feat(part), s] for the
     out-projection; both heads of a TP... head-pair share one PSUM
     transpose tile, drained by one DVE copy.
  D. out_partial = O @ Wout_slice via lhsT=oT chunks, written [s, 768] fp32.

  Emission order is driven by a greedy scheduler: attention jobs emit
  k-group by k-group (each group = 2 S^T matmuls + 1 batched exp + the
  previous group's masked PV flush); between groups, PE-heavy "filler"
  units (qkv f-tiles, V s-tiles, out-proj halves, transpose batches) are
  pulled from a queue whenever modeled cumulative PE work falls behind
  modeled ACT work + LEAD.  This keeps the Tensor engine continuously
  busy (and therefore at the fast p-state) while ACT grinds through the
  exps, which are the second-largest engine load.
"""
from collections import deque

import numpy as np
import concourse.bass as bass
import concourse.mybir as mybir
import concourse.tile as tile
from concourse import bacc
from concourse.bass_utils import run_bass_kernel_spmd
from concourse.masks import make_identity

B, S, D = 4, 2048, 768
H, HD = 12, 64
N_CORES = 8
HPC = H // 2          # heads per core = 6
FQK = HPC * HD        # 384 features per core for each of q,k,v
F32 = mybir.dt.float32
F16 = mybir.dt.float16

N_ST = S // 128       # 16 s tiles
N_QC = S // 512       # 4 q chunks
N_DT = D // 128       # 6 d_model tiles

PE_C = 1.0 / 2.4      # ns per PE row at full clock (cost model)
ACT_C = 1.0 / 1.2     # ns per ACT column

TRACE = False         # set by test.py for profiling runs
DEBUG = False         # adds intermediate-dump DRAM outputs
_CACHE = {}
PHASE_MARKS = []      # (phase_name, first_inst_id) - filled during _emit
EMIT_STATS = {}       # modeled clocks, for offline schedule debugging
STARVE_LOG = []       # (job, ns) filler-bank dry spells during emission
CUR_JOB = ["init"]

# job order: head-pairs adjacent per qc (for shared transpose tiles);
# ACT-heavy (high-qc) jobs interleaved with PE-rich stretches so the
# filler bank (qkv f-tiles / V tiles / projections) never runs dry
JOB_ORDER = [
    (0, 0), (1, 0), (0, 1), (1, 1),
    (0, 2), (1, 2), (0, 3), (1, 3),
    (2, 2), (4, 2), (3, 2), (5, 2),
    (2, 3), (4, 3), (3, 3), (5, 3),
    (2, 1), (4, 1), (3, 1), (5, 1),
    (2, 0), (4, 0), (3, 0), (5, 0),
]


def _mark(nc, name):
    PHASE_MARKS.append((name, nc.next_id()))


def _emit(nc):
    xt_d = nc.dram_tensor("xt", [D, S], F16, kind="ExternalInput").ap()
    wqkv_d = nc.dram_tensor("wqkv", [D, 3 * FQK], F16, kind="ExternalInput").ap()
    bqk_d = nc.dram_tensor("bqk", [128, 6], F32, kind="ExternalInput").ap()
    vb_d = nc.dram_tensor("vb", [128, FQK], F32, kind="ExternalInput").ap()
    wout_d = nc.dram_tensor("wout", [FQK, D], F16, kind="ExternalInput").ap()
    out_d = nc.dram_tensor("out", [S, D], F32, kind="ExternalOutput").ap()
    if DEBUG:
        dbg_qkT = nc.dram_tensor("dbg_qkT", [128, 6, S], F16,
                                 kind="ExternalOutput").ap()
        dbg_vn = nc.dram_tensor("dbg_vn", [128, N_ST, HPC, HD + 1], F16,
                                kind="ExternalOutput").ap()
        dbg_oT = nc.dram_tensor("dbg_oT", [128, FQK // 128, S], F16,
                                kind="ExternalOutput").ap()

    with tile.TileContext(nc) as tc:
        with tc.tile_pool(name="const", bufs=1) as pc, \
             tc.tile_pool(name="xT", bufs=1) as pxt, \
             tc.tile_pool(name="qkT", bufs=1) as pqk, \
             tc.tile_pool(name="vn", bufs=1) as pvn, \
             tc.tile_pool(name="wq", bufs=1) as pwq, \
             tc.tile_pool(name="OT", bufs=1) as pot, \
             tc.tile_pool(name="pt", bufs=4) as ppt, \
             tc.tile_pool(name="on", bufs=8) as pon, \
             tc.tile_pool(name="rc", bufs=8) as prc, \
             tc.tile_pool(name="outp", bufs=2) as pout, \
             tc.tile_pool(name="ps", bufs=2, space="PSUM") as pp, \
             tc.tile_pool(name="pso", bufs=2, space="PSUM") as ppo, \
             tc.tile_pool(name="aux", bufs=2, space="PSUM") as paux:

            bqk_sb = pc.tile([128, 6], F32)
            vb_sb = pc.tile([128, FQK], F32)
            ident = pc.tile([128, 128], F16)
            make_identity(nc, ident[:])
            # zero operand for the PV group-opening matmul (one PSUM bank may
            # hold only ONE open accumulation group: interleaved per-region
            # start flags corrupt each other's partial sums on hardware)
            zeros = pc.tile([128, 4 * (HD + 1)], F16)
            nc.vector.memset(zeros[:], 0.0)

            xT = pxt.tile([128, N_DT, S], F16)
            qkT = pqk.tile([128, 6, S], F16)
            # Vn: [s(part), s_tile, head, 65] with ones col at 64
            vn = pvn.tile([128, N_ST, HPC, HD + 1], F16)
            wqkv_sb = pwq.tile([128, N_DT, 3 * FQK], F16)
            wout_sb = pwq.tile([128, FQK // 128, D], F16)
            oT = pot.tile([128, FQK // 128, S], F16)

            nc.vector.memset(vn[:, :, :, HD:HD + 1], 1.0)

            # ---- input DMAs, priority-ordered (HWDGE generates in order;
            # tile subtile-deps gate the first consumer of each slice) ----
            _mark(nc, "A:dma")

            def dma_w(ft, lo=0, hi=N_DT):
                nc.sync.dma_start(
                    wqkv_sb[:, lo:hi, ft * 128:(ft + 1) * 128],
                    wqkv_d[lo * 128:hi * 128,
                           ft * 128:(ft + 1) * 128].rearrange(
                        "(t p) f -> p t f", p=128))

            def dma_x(sc):
                for dc in range(N_DT):
                    nc.sync.dma_start(
                        xT[:, dc, sc * 512:(sc + 1) * 512],
                        xt_d[dc * 128:(dc + 1) * 128, sc * 512:(sc + 1) * 512])

            dma_w(0, 0, 3)     # split first weight tile: PE starts sooner
            dma_w(0, 3, 6)
            nc.sync.dma_start(bqk_sb[:], bqk_d[:])
            dma_x(0)
            dma_w(3)
            nc.sync.dma_start(vb_sb[:], vb_d[:])
            nc.sync.dma_start(   # wv before xT sc1: first V unit needs it
                wqkv_sb[:, :, 2 * FQK:],
                wqkv_d[:, 2 * FQK:].rearrange("(t p) f -> p t f", p=128))
            dma_x(1)
            dma_x(2)
            dma_w(1)
            dma_w(4)
            dma_x(3)
            dma_w(2)
            dma_w(5)
            nc.sync.dma_start(
                wout_sb[:], wout_d.rearrange("(t p) o -> p t o", p=128))

            vb_h = vb_sb.rearrange("p (h d) -> p h d", d=HD)

            # ---- virtual-clock list scheduler: pe_t / act_t track the
            # modeled finish time of all issued PE / ACT work.  Before a PV
            # flush (which needs its group's exp done), fillers are emitted
            # until PE's frontier covers the exp-ready time, so PE never
            # idles waiting on ACT. ----
            clk = {"pe": 0.0, "act": 0.0, "starve": 0.0, "last_proj": -1e9}
            fillers = deque()     # keys, FIFO
            filler_fns = {}       # key -> (pe_cost, fn)
            filler_ready = {}     # key -> earliest clk.pe this unit can run
            emitted = set()
            o_n_store = {}        # (h, qc) -> normalized O sbuf tile
            pending_trs = []      # registered-not-yet-emitted tr keys

            def register(key, cost, fn, ready=0.0):
                filler_fns[key] = (cost, fn)
                filler_ready[key] = ready
                fillers.append(key)

            def force(key):
                if key in emitted:
                    return
                emitted.add(key)
                cost, fn = filler_fns[key]
                fn()
                clk["pe"] += cost

            def fill_until(t):
                if clk.get("filling"):
                    return
                clk["filling"] = True
                try:
                    _fill_until(t)
                finally:
                    clk["filling"] = False

            def _fill_until(t):
                scanned = 0
                while fillers and clk["pe"] < t and scanned < len(fillers):
                    key = fillers.popleft()
                    if key in emitted:
                        continue
                    rdy = filler_ready.get(key, 0.0)
                    if rdy > clk["pe"]:
                        if rdy < t and scanned + 1 >= len(fillers):
                            # would starve anyway: jump to its ready time
                            clk["pe"] = rdy
                            force(key)
                            scanned = 0
                            continue
                        fillers.append(key)   # not ready yet; rotate
                        scanned += 1
                        continue
                    scanned = 0
                    force(key)
                if clk["pe"] < t:
                    clk["starve"] += t - clk["pe"]
                    STARVE_LOG.append((CUR_JOB[0], round(t - clk["pe"]),
                                       [k for k in fillers
                                        if k not in emitted]))

            # PSUM slots pace emission: allocating one waits (in hardware)
            # for its previous tenant's consumer to drain, so model each
            # slot ring's free times and pour filler into score-slot waits.
            # Scores own the "s" ring; filler/proj/transpose units share
            # the 2-slot "aux" ring so they never stall the score stream.
            s_free = deque([0.0, 0.0])
            aux_free = deque([0.0, 0.0])

            def s_gate():
                gate = s_free.popleft()
                fill_until(gate)
                clk["pe"] = max(clk["pe"], gate)

            def aux_gate():
                gate = aux_free.popleft()
                clk["pe"] = max(clk["pe"], gate)

            # ---- filler units (fine-grained so dep blobs stay small) ----
            def emit_v(st):
                aux_gate()
                ps_v = paux.tile([128, FQK], F32, tag="aux")
                for dc in range(N_DT):
                    nc.tensor.matmul(
                        ps_v[:, :],
                        xT[:, dc, st * 128:(st + 1) * 128],
                        wqkv_sb[:, dc, 2 * FQK:],
                        start=(dc == 0), stop=(dc == N_DT - 1))
                nc.vector.tensor_tensor(
                    vn[:, st, :, 0:HD],
                    ps_v[:, :].rearrange("p (h d) -> p h d", d=HD),
                    vb_h, mybir.AluOpType.add)
                aux_free.append(clk["pe"] + 6 * FQK * PE_C + 800.0)

            def emit_ft(ft, sc):
                aux_gate()
                ps_qk = paux.tile([128, 512], F32, tag="aux")
                for dc in range(N_DT):
                    nc.tensor.matmul(
                        ps_qk[:, :],
                        wqkv_sb[:, dc, ft * 128:(ft + 1) * 128],
                        xT[:, dc, sc * 512:(sc + 1) * 512],
                        start=(dc == 0), stop=(dc == N_DT - 1))
                nc.vector.tensor_scalar(
                    qkT[:, ft, sc * 512:(sc + 1) * 512],
                    ps_qk[:, :],
                    bqk_sb[:, ft:ft + 1], None, mybir.AluOpType.add)
                aux_free.append(clk["pe"] + 6 * 512 * PE_C + 800.0)

            def emit_tr(pair, qc):
                aux_gate()
                tr = paux.tile([128, 512], F16, tag="aux", name="tr")
                for hh in (2 * pair, 2 * pair + 1):
                    po = (hh % 2) * 64
                    o_n = o_n_store.pop((hh, qc))
                    for qt in range(4):
                        nc.tensor.transpose(
                            tr[po:po + 64, qt * 128:(qt + 1) * 128],
                            o_n[:, qt, :], ident[:])
                nc.vector.tensor_copy(oT[:, pair, qc * 512:(qc + 1) * 512],
                                      tr[:])
                aux_free.append(clk["pe"] + 8 * 128 * PE_C + 800.0)

            proj_osb = {}

            def emit_proj(st, oc, tail=False):
                # oT for this st's q-chunk must be WRITTEN (emission order
                # = dependency order): force the three transpose units
                for pair in range(3):
                    key = ("tr", pair, st // 4)
                    if key in filler_fns:
                        force(key)
                if oc == 0:
                    proj_osb[st] = pout.tile([128, D], F32, tag="osb",
                                             name="o_sb")
                o_sb = proj_osb[st]
                if tail:
                    ps_big = pp.tile([128, 1024], F32, tag="s")
                    ps_d = ps_big[:, oc * 512:oc * 512 + 384]
                else:
                    aux_gate()
                    ps_d = paux.tile([128, 384], F32, tag="aux", name="ps_d")
                for ht in range(FQK // 128):
                    nc.tensor.matmul(
                        ps_d[:, :],
                        oT[:, ht, st * 128:(st + 1) * 128],
                        wout_sb[:, ht, oc * 384:(oc + 1) * 384],
                        start=(ht == 0), stop=(ht == FQK // 128 - 1))
                nc.vector.tensor_copy(o_sb[:, oc * 384:(oc + 1) * 384],
                                      ps_d[:, :])
                if not tail:
                    aux_free.append(clk["pe"] + 3 * 384 * PE_C + 700.0)
                    if oc == 1:
                        nc.sync.dma_start(out_d[st * 128:(st + 1) * 128, :],
                                          o_sb[:])
                        del proj_osb[st]
                else:
                    # tail: ship each half as soon as it is staged so the
                    # final drain only waits on the last 384-column DMA
                    nc.sync.dma_start(
                        out_d[st * 128:(st + 1) * 128,
                              oc * 384:(oc + 1) * 384],
                        o_sb[:, oc * 384:(oc + 1) * 384])
                    if oc == 1:
                        del proj_osb[st]

            for _ft in range(6):
                for _sc in range(4):
                    register(("ft", _ft, _sc), 6 * 512 * PE_C,
                             lambda f=_ft, s=_sc: emit_ft(f, s))
            for _st in range(N_ST):
                register(("v", _st), 6 * FQK * PE_C,
                         lambda s=_st: emit_v(s))

            # initial queue order: first-use-ish; later heads' weight tiles
            # and V tiles trail so they stay banked for the ACT-heavy midgame
            fillers.clear()
            for key in ([("ft", 0, 0), ("ft", 3, 0), ("v", 0), ("v", 1),
                         ("ft", 0, 1), ("ft", 3, 1), ("v", 2), ("v", 3)]
                        + [("ft", f, s) for s in range(2)
                           for f in (1, 4, 2, 5)]
                        + [("v", _st) for _st in range(4, 10)]
                        + [("ft", f, s) for s in range(2, 4)
                           for f in (0, 3, 1, 4, 2, 5)]
                        + [("v", _st) for _st in range(10, 16)]):
                fillers.append(key)

            def prioritize(key):
                if key in emitted or key not in filler_fns:
                    return
                try:
                    fillers.remove(key)
                except ValueError:
                    pass
                fillers.appendleft(key)

            # ---- attention job: k-loop with one-group PV deferral; the
            # job's final group flushes inside the NEXT job (pend) ----
            pend = {"t": None}
            normed = {qc: 0 for qc in range(N_QC)}

            def note_normed(h, qc):
                normed[qc] += 1
                if h % 2 == 1:
                    pair = h // 2
                    # small ready-time keeps the PE transposes from being
                    # pulled before the DVE normalize chain produced o_n
                    register(("tr", pair, qc), 8 * 128 * PE_C,
                             lambda p=pair, q=qc: emit_tr(p, q),
                             ready=clk["pe"] + 500.0)
                    pending_trs.append(("tr", pair, qc))
                if normed[qc] == HPC:
                    for st in range(qc * 4, qc * 4 + 4):
                        for oc in range(2):
                            register(("proj", st, oc), 3 * 384 * PE_C,
                                     lambda s=st, o=oc: emit_proj(s, o))

            def force_deps(h, qc):
                # q f_tile chunk qc; k chunks are forced JIT in the k-loop
                force(("ft", h // 2, qc))
                force(("ft", 3 + h // 2, 0))

            def attn_job(h, qc):
                po = (h % 2) * 64
                qf = h // 2
                kf = 3 + h // 2
                # bound pending transpose units so o_n slots can't exhaust
                # behind a blocked PE instruction (deadlock guard)
                pending_trs[:] = [k for k in pending_trs if k not in emitted]
                while len(pending_trs) > 2:
                    force(pending_trs.pop(0))
                force_deps(h, qc)
                ps_o = ppo.tile([128, 4, HD + 1], F32, tag="o")
                n_kt = 4 * (qc + 1)
                # open ONE accumulation group for the whole bank, zeroing all
                # four qt regions; every PV then accumulates with start=False
                nc.tensor.matmul(ps_o[:, :, :], ident[:], zeros[:],
                                 start=True, stop=False)
                clk["pe"] += 4 * (HD + 1) * PE_C

                def flush(ktg, offs, pt, ready, last):
                    fill_until(ready)
                    clk["pe"] = max(clk["pe"], ready)
                    for j in range(2):
                        kt = ktg + j
                        q_off = offs[j]
                        force(("v", kt))
                        for qt in range(4):
                            if kt <= qc * 4 + qt:
                                nc.tensor.matmul(
                                    ps_o[:, qt, :],
                                    pt[:, j * 512 + qt * 128:
                                       j * 512 + (qt + 1) * 128],
                                    vn[:, kt, h, :],
                                    start=False,
                                    stop=(kt == n_kt - 1 and qt == 3))
                                clk["pe"] += 65 * PE_C
                    if last:
                        recip = prc.tile([128, 4, 1], F32, tag="rc")
                        nc.vector.reciprocal(recip[:], ps_o[:, :, HD:HD + 1])
                        o_n = pon.tile([128, 4, HD], F16, tag="on")
                        for qt in range(4):
                            nc.vector.tensor_scalar(
                                o_n[:, qt, :], ps_o[:, qt, 0:HD],
                                recip[:, qt, :], None, mybir.AluOpType.mult)
                        o_n_store[(h, qc)] = o_n
                        note_normed(h, qc)

                unflushed = deque()
                for ktg in range(0, n_kt, 2):
                    force(("ft", kf, ktg // 4))
                    force(("ft", kf, (ktg + 1) // 4))
                    s_gate()
                    ps_s = pp.tile([128, 1024], F32, tag="s")
                    offs = []
                    for j in range(2):
                        kt = ktg + j
                        q_off = max(0, kt * 128 - qc * 512)
                        offs.append(q_off)
                        nc.tensor.matmul(
                            ps_s[:, j * 512 + q_off:(j + 1) * 512],
                            qkT[po:po + 64, kf, kt * 128:(kt + 1) * 128],
                            qkT[po:po + 64, qf,
                                qc * 512 + q_off:(qc + 1) * 512],
                            start=True, stop=True)
                        clk["pe"] += (512 - q_off) * PE_C
                    pt = ppt.tile([128, 1024], F16, tag="pt")
                    if offs[0] >= 256:
                        # deep-diagonal pair: separate exps skip the garbage
                        # columns between the two tiles' valid ranges
                        for j in range(2):
                            nc.scalar.activation(
                                pt[:, j * 512 + offs[j]:(j + 1) * 512],
                                ps_s[:, j * 512 + offs[j]:(j + 1) * 512],
                                mybir.ActivationFunctionType.Exp)
                        cols = (512 - offs[0]) + (512 - offs[1])
                        clk["act"] = (max(clk["act"], clk["pe"] + 150)
                                      + cols * ACT_C + 2 * 185)
                    else:
                        nc.scalar.activation(
                            pt[:, offs[0]:], ps_s[:, offs[0]:],
                            mybir.ActivationFunctionType.Exp)
                        clk["act"] = (max(clk["act"], clk["pe"] + 150)
                                      + (1024 - offs[0]) * ACT_C + 185)
                    s_free.append(clk["act"])
                    # causal mask for diagonal tiles, emitted right after the
                    # exp so the (idle) Pool engine applies it long before the
                    # deferred PV flush reads pt
                    for j in range(2):
                        kt = ktg + j
                        if kt * 128 >= qc * 512:
                            q_off = offs[j]
                            sl = slice(j * 512 + q_off, j * 512 + q_off + 128)
                            nc.gpsimd.affine_select(
                                out=pt[:, sl], in_=pt[:, sl],
                                compare_op=mybir.AluOpType.is_ge,
                                fill=0.0, base=0, channel_multiplier=-1,
                                pattern=[[1, 128]])
                    ready = clk["act"] + 150.0
                    if ktg == 0 and pend["t"] is not None:
                        pend["t"]()  # prev job's final PVs + its normalize
                        pend["t"] = None
                    unflushed.append((ktg, offs, pt, ready))
                    if len(unflushed) > 2:
                        flush(*unflushed.popleft(), last=False)
                while len(unflushed) > 1:
                    flush(*unflushed.popleft(), last=False)
                pend["t"] = lambda pv=unflushed.popleft(): flush(*pv,
                                                                 last=True)

            for i, (h, qc) in enumerate(JOB_ORDER):
                _mark(nc, f"C:h{h} qc{qc}")
                CUR_JOB[0] = f"h{h}q{qc}"
                if i + 1 < len(JOB_ORDER):
                    # float the next job's dep tiles to the queue front so
                    # they get pulled as filler during THIS job instead of
                    # landing as an ACT-stalling blob at the boundary
                    nh, nqc = JOB_ORDER[i + 1]
                    for key in [("ft", 3 + nh // 2, sc)
                                for sc in range(nqc, -1, -1)] + \
                               [("ft", nh // 2, nqc)]:
                        prioritize(key)
                attn_job(h, qc)

            _mark(nc, "D:tail")
            CUR_JOB[0] = "tail"
            pend["t"]()  # final job's deferred PV group + normalize
            pend["t"] = None
            # drain every remaining filler; projs not yet emitted use the
            # (now free) score-psum banks so halves never serialize
            for pair in range(3):
                for qc in range(N_QC):
                    if ("tr", pair, qc) in filler_fns:
                        force(("tr", pair, qc))
            for st in range(N_ST):
                for oc in range(2):
                    key = ("proj", st, oc)
                    if key in filler_fns and key not in emitted:
                        emitted.add(key)
                        emit_proj(st, oc, tail=True)
            # anything else (ft/v) must already be in; assert coverage
            for key in filler_fns:
                assert key in emitted, f"filler {key} never emitted"
            if DEBUG:
                nc.sync.dma_start(dbg_qkT[:], qkT[:])
                nc.sync.dma_start(dbg_vn[:], vn[:])
                nc.sync.dma_start(dbg_oT[:], oT[:])
            EMIT_STATS.update(clk)


def _build():
    if "nc" not in _CACHE:
        nc = bacc.Bacc("TRN2", target_bir_lowering=False, debug=False,
                       num_devices=N_CORES)
        _emit(nc)
        nc.compile()
        _CACHE["nc"] = nc
    return _CACHE["nc"]


def kernel(x, qkv_w, qkv_b, out_w, out_b):
    x = np.asarray(x, dtype=np.float32)
    qkv_w = np.asarray(qkv_w, dtype=np.float32)
    qkv_b = np.asarray(qkv_b, dtype=np.float32)
    out_w = np.asarray(out_w, dtype=np.float32)
    out_b = np.asarray(out_b, dtype=np.float32)

    nc = _build()
    scale = HD ** -0.5
    in_maps = []
    for c in range(N_CORES):
        b, half = c // 2, c % 2
        fq = slice(half * FQK, (half + 1) * FQK)
        fk = slice(D + half * FQK, D + (half + 1) * FQK)
        fv = slice(2 * D + half * FQK, 2 * D + (half + 1) * FQK)
        wq = qkv_w[:, fq] * scale
        wk = qkv_w[:, fk]
        wv = qkv_w[:, fv]
        wqkv = np.ascontiguousarray(
            np.concatenate([wq, wk, wv], axis=1), dtype=np.float16)
        bqk = np.concatenate([qkv_b[fq] * scale, qkv_b[fk]])  # [768]
        bqk = np.ascontiguousarray(
            bqk.reshape(6, 128).T, dtype=np.float32)          # [128, 6]
        vb = np.ascontiguousarray(
            np.broadcast_to(qkv_b[fv], (128, FQK)), dtype=np.float32)
        wout = np.ascontiguousarray(
            out_w[half * FQK:(half + 1) * FQK, :], dtype=np.float16)
        in_maps.append({
            "xt": np.ascontiguousarray(x[b].T).astype(np.float16),
            "wqkv": wqkv, "bqk": bqk, "vb": vb, "wout": wout,
        })

    res = run_bass_kernel_spmd(nc, in_maps, list(range(N_CORES)), trace=TRACE)
    parts = [res.results[c]["out"] for c in range(N_CORES)]
    out = np.empty((B, S, D), dtype=np.float32)
    for b in range(B):
        out[b] = parts[2 * b] + parts[2 * b + 1] + out_b
    if TRACE:
        kernel.last_results = res
    return out

